# revision 31
# baseline (speedup 1.0000x reference)
"""Multi-head causal attention on 8 Trainium2 NeuronCores.

Sharding: tensor-parallel over heads (2 heads/core) for QKV projection and
attention; All-to-All converts to token-sharding (512 tokens/core) for the
output projection, so each core writes a disjoint output slice and the host
gather is pure concatenation.

v2 layout strategy (per core), all-bf16 data plane:
  - qT/kT = W^T x^T computed in transposed [feature, token] layout (x^T in
    bf16 prepared on host); V computed directly in token-major layout with
    x^T tiles as the stationary operand (no PE transposes).
  - scores^T[k, q] = K_tile^T.T @ Q^T, two heads row-tiled on the PE.
    Causal diagonal tiles shrink to their valid q-suffix (saves ~15% PE+ACT),
    with one [128,128] triangular bf16 mask for the boundary block.
  - softmax: exp on ScalarE out of PSUM with 1/sqrt(D) folded into the
    activation scale; denominator via a ones-column appended to V.
  - normalization: reciprocal (DVE) -> partition_broadcast + multiply on the
    otherwise-idle GpSimd engine (no DRAM round trip, no DVE head-of-line
    blocking).
  - A2A payload bf16 (halves link traffic); output projection streams Wo
    (preloaded early) against per-row-DMA'd A2A results, PSUM -> DRAM direct.
"""

import numpy as np
import ml_dtypes

import concourse.bass as bass
import concourse.mybir as mybir
import concourse.tile as tile
from concourse.bass_utils import run_bass_kernel_spmd
from concourse.masks import make_identity

F32 = mybir.dt.float32
BF16 = mybir.dt.bfloat16
AF = mybir.ActivationFunctionType


def _install_cache_nonce_hook():
    """The libneuronxla NEFF cache hashes the HLO but the BIR rides in
    backend_config (excluded from the hash), so edited kernels with the same
    I/O signature can silently hit a stale cached NEFF. Inject a hash of the
    BIR into mhlo.frontend_attributes — which IS part of the model hash —
    the same way bass2jax ships the DVE tables."""
    import hashlib
    import concourse.bass2jax as bass2jax
    from jax.interpreters import mlir

    if getattr(bass2jax, "_ant_cache_nonce_hooked", False):
        return
    bass2jax._ant_cache_nonce_hooked = True
    orig = bass2jax._accumulate_module_dve_attrs

    def patched(ctx, nc):
        orig(ctx, nc)
        op = ctx.module_context.module.operation
        cur = (
            op.attributes["mhlo.frontend_attributes"]
            if "mhlo.frontend_attributes" in op.attributes
            else None
        )
        existing = (
            {a.name: mlir.ir.StringAttr(a.attr).value for a in cur}
            if cur is not None
            else {}
        )
        existing["ant.cache_nonce"] = hashlib.sha256(
            nc.to_json_bytes()
        ).hexdigest()
        op.attributes["mhlo.frontend_attributes"] = mlir.ir.DictAttr.get(
            {k: mlir.ir.StringAttr.get(v) for k, v in existing.items()}
        )

    bass2jax._accumulate_module_dve_attrs = patched


_install_cache_nonce_hook()


def _install_ldw_opt_hook():
    """bass_utils hardcodes --enable-ldw-opt=false. Flipping it to true fails
    in THIS walrus build even for a minimal all-bf16 single-matmul kernel
    ("InstLdweights is not compatible with LDW optimization" from
    visitInstLdweights) — the pass expects to synthesize Ldweights itself and
    rejects bass's explicit ones. Kept for reference; do NOT install."""
    import concourse.bass_utils as bu

    if getattr(bu, "_ant_ldw_opt_hooked", False):
        return
    bu._ant_ldw_opt_hooked = True
    orig = bu.run_command

    def patched(argv, **kwargs):
        argv = [
            "--enable-ldw-opt=true" if a == "--enable-ldw-opt=false" else a
            for a in argv
        ]
        return orig(argv, **kwargs)

    bu.run_command = patched


B, S, DM = 2, 2048, 1024
H, D = 16, 64
NCORES = 8
HP = H // NCORES          # heads per core
T = B * S                 # 4096 tokens
TCHUNK = T // NCORES      # 512 tokens per a2a chunk
NCH = T // 512            # 8 token chunks of 512
KT_PER_S = S // 128       # 16 k-tiles per sequence
QT_PER_S = S // 512       # 4 q-tiles per sequence
SCALE = 1.0 / np.sqrt(D)


MAX_WAITS = 1  # walrus in this container rejects >1 sem-wait per instruction


def _split_waits(nc, limit=MAX_WAITS):
    """Post-pass: move excess sem-waits onto preceding same-engine nops.

    Engines dispatch in program order and a sem-wait stalls the engine's NX
    before anything later is enqueued, so carrying the waits on nops placed
    immediately before the instruction is semantically identical.
    """
    n_id = 0
    for bb in nc.main_func.blocks:
        new = []
        for inst in bb.instructions:
            si = getattr(inst, "sync_info", None)
            # walrus --enable-ldw-opt fuses each Ldweights into the previous
            # matmul's pipeline and rejects any Ldweights carrying sem-waits;
            # move ALL of its waits onto nops (semantically identical: the
            # engine stalls just before instead of at the Ldweights).
            keep = 0 if isinstance(inst, mybir.InstLdweights) else limit
            if si is not None and len(si.on_wait) > keep:
                waits = list(si.on_wait)
                kept = waits[len(waits) - keep :] if keep else []
                move = waits[: len(waits) - keep]
                for i in range(0, len(move), limit):
                    nop = mybir.InstNoOp(
                        name=f"wsplit-{n_id}", ins=[], outs=[], engine=inst.engine
                    )
                    n_id += 1
                    nop.sync_info = mybir.SyncInfo(
                        on_wait=move[i : i + limit], on_update=[]
                    )
                    new.append(nop)
                inst.sync_info = mybir.SyncInfo(
                    on_wait=kept, on_update=list(si.on_update)
                )
            new.append(inst)
        bb.instructions = new


from concourse.vector_clock import ScopedClock


class _TileCtx(tile.TileContext):
    """Work around a walrus codegen limit: the stock tail drain carries one
    sem-wait per (engine, DMA-lane), but this compiler build rejects >1-2
    waits on a Drain ("Too many sync wait commands"). Put each wait on its
    own SP nop between the drain and the final barrier instead."""

    def _drain_and_barrier(self, tick_clock, wait_clock):
        nc = self.nc
        drain_inst = nc.sync.drain()
        wait_clock.add_sem_waits(
            drain_inst.ins, ScopedClock({None: tick_clock.global_clock})
        )
        si = drain_inst.ins.sync_info
        if si is not None and len(si.on_wait) > 1:
            waits = list(si.on_wait)
            drain_inst.ins.sync_info = mybir.SyncInfo(
                on_wait=[waits[0]], on_update=list(si.on_update)
            )
            for w in waits[1:]:
                nop = nc.sync.nop(nofuse=True, hint="tail_drain_wait_split")
                nop.ins.sync_info = mybir.SyncInfo(on_wait=[w], on_update=[])

        nc.all_engine_barrier()
        assert self.sems is not None
        popped = nc._tile_sem_poison_stack.pop()
        assert popped is self._sem_poison
        nc.clear_and_free_semaphores(list(self.sems.allocated().values()))
        nc.all_engine_barrier()


def _kt_seq(qt, mode):
    """K-tile processing order for q-tile qt.

    Causal: diagonal group first (the o=0 member is full-width, so it can
    carry start=True for the whole-AV accumulation), then the earlier full
    tiles. Returns list of (kt, off) where off is the first valid q column.
    """
    if mode == "causal":
        seq = [(4 * qt + o, 128 * o) for o in range(4)]
        seq += [(kt, 0) for kt in range(4 * qt)]
        return seq
    return [(kt, 0) for kt in range(KT_PER_S)]


def build(mode, n_mask_tiles, skip_phase3=False):
    """Build the SPMD Bass program. mode: 'causal' | 'full' | 'general'."""
    nc = bass.Bass()

    xT = nc.dram_tensor("xT", [DM, T], BF16, kind="ExternalInput")
    wq = nc.dram_tensor("wq", [DM, 128], BF16, kind="ExternalInput")
    wk = nc.dram_tensor("wk", [DM, 128], BF16, kind="ExternalInput")
    wv = nc.dram_tensor("wv", [DM, 128], BF16, kind="ExternalInput")
    wo = nc.dram_tensor("wo", [DM, DM], BF16, kind="ExternalInput")
    if mode == "causal":
        tri = nc.dram_tensor("tri", [128, 128], BF16, kind="ExternalInput")
    if n_mask_tiles:
        mt = nc.dram_tensor(
            "mt", [n_mask_tiles, 128, 512], BF16, kind="ExternalInput"
        )
    out = nc.dram_tensor("out", [TCHUNK, DM], F32, kind="ExternalOutput")

    with _TileCtx(nc) as tc:
        with (
            tc.tile_pool(name="const", bufs=1) as const,
            tc.tile_pool(name="xin", bufs=1) as xin,
            tc.tile_pool(name="pp", bufs=6) as pp,
            tc.tile_pool(name="misc", bufs=4) as misc,
            tc.tile_pool(name="psS", bufs=2, space="PSUM") as psS,
            tc.tile_pool(name="psAV", bufs=2, space="PSUM") as psAV,
            tc.tile_pool(name="psT", bufs=2, space="PSUM") as psT,
            tc.tile_pool(name="dram", bufs=1, space="DRAM") as dram,
        ):
            # ---- resident SBUF tensors ----
            wq_sb = const.tile([128, 8, 128], BF16)
            wk_sb = const.tile([128, 8, 128], BF16)
            wv_sb = const.tile([128, 8, 128], BF16)
            for w_sb, w in ((wq_sb, wq), (wk_sb, wk), (wv_sb, wv)):
                src = w.rearrange("(o p) e -> p o e", p=128)
                nc.sync.dma_start(w_sb[:, 0:4, :], src[:, 0:4, :])
                nc.sync.dma_start(w_sb[:, 4:8, :], src[:, 4:8, :])

            if mode == "causal":
                tri_sb = const.tile([128, 128], BF16)
                nc.sync.dma_start(tri_sb[:], tri[:, :])
            if n_mask_tiles:
                mt_sb = const.tile([128, n_mask_tiles, 512], BF16)
                nc.sync.dma_start(mt_sb[:], mt.rearrange("m p q -> p m q"))

            qT_sb = const.tile([128, NCH, 512], BF16)
            kT_sb = const.tile([128, NCH, 512], BF16)
            # V in [token, feature] layout, per k-tile, per head:
            # [p=token%128, ktile, head, 80] where cols 0:64 = v, col 64 = 1.0
            v_sb = const.tile([128, T // 128, HP, 80], BF16)
            nc.vector.memset(v_sb[:, :, :, 64:65], 1.0)
            wo_sb = const.tile([128, 8, DM], BF16)
            ones_sb = const.tile([1, 64], BF16)
            nc.vector.memset(ones_sb[:], 1.0)
            ident = const.tile([128, 128], F32)
            make_identity(nc, ident[:])

            a2a_in = [
                dram.tile([NCORES, 128, 256], BF16, name=f"a2a_in{b}")
                for b in range(B)
            ]
            a2a_out = [
                dram.tile([NCORES, 128, 256], BF16, name=f"a2a_out{b}")
                for b in range(B)
            ]

            # all of x^T preloaded up front (fits SBUF at bf16 when no mask
            # tiles are resident): the DMA burst is concentrated at kernel
            # start instead of taxing the PE's SBUF reads throughout the
            # whole QKV phase.
            preload = not n_mask_tiles
            xts = [None] * NCH

            def load_xt(c):
                xt = xin.tile(
                    [128, 8, 512], BF16,
                    tag=f"xt{c}" if preload else "xt",
                    bufs=1 if preload else 2,
                    name=f"xt{c}",
                )
                src = xT[:, 512 * c : 512 * (c + 1)].rearrange(
                    "(o p) s -> p o s", p=128
                )
                nc.sync.dma_start(xt[:, 0:4, :], src[:, 0:4, :])
                nc.sync.dma_start(xt[:, 4:8, :], src[:, 4:8, :])
                return xt

            if preload:
                for c in range(NCH):
                    xts[c] = load_xt(c)

            def qkv_chunk(c):
                xt = xts[c] if preload else load_xt(c)
                # q, k, v: weight-stationary, [feature, token] layout.
                # (One 512-wide matmul per contraction step beats a token-major
                # V: the PE is instruction-rate-bound, not row-bound.)
                for name, w_sb, dst in (
                    ("q", wq_sb, qT_sb),
                    ("k", wk_sb, kT_sb),
                    ("v", wv_sb, None),
                ):
                    psum = psT.tile(
                        [128, 512], F32, tag="t", name=f"ps_{name}{c}"
                    )
                    for kt in range(8):
                        nc.tensor.matmul(
                            psum[:],
                            w_sb[:, kt, :],
                            xt[:, kt, :],
                            start=(kt == 0),
                            stop=(kt == 7),
                        )
                    if dst is not None:
                        nc.vector.tensor_copy(dst[:, c, :], psum[:])
                        continue
                    # V: PE-transpose [feature, token] -> [token, feature].
                    # f32 throughout: a bf16 PSUM transpose output compiles
                    # but takes down the exec unit at runtime. Copies run as
                    # ScalarE activation-copies (Pool can't read PSUM) so the
                    # in-order DVE (busy with q/k copies + attention
                    # epilogues) never gates the psT ring.
                    vstg = misc.tile([128, 512], F32, tag="vstg", bufs=2)
                    nc.scalar.activation(vstg[:], psum[:], AF.Copy)
                    ps_t = psT.tile([128, 512], F32, tag="t", name=f"ps_t{c}")
                    for sub in range(4):
                        nc.tensor.transpose(
                            ps_t[:, 128 * sub : 128 * (sub + 1)],
                            vstg[:, 128 * sub : 128 * (sub + 1)],
                            ident[:],
                        )
                    for sub in range(4):
                        ktile = 4 * c + sub
                        for h in range(HP):
                            nc.scalar.activation(
                                v_sb[:, ktile, h, 0:64],
                                ps_t[
                                    :,
                                    128 * sub + 64 * h : 128 * sub
                                    + 64 * (h + 1),
                                ],
                                AF.Copy,
                            )

            def attention(b, qt):
                ch = b * QT_PER_S + qt
                seq = _kt_seq(qt, mode)
                av = [
                    psAV.tile([128, 512], F32, tag="av", name=f"av{ch}_{h}")
                    for h in range(HP)
                ]

                def emit_scores(kt, off):
                    # both heads share one [128, 2, 512] PSUM tile so a single
                    # ScalarE exp covers them (ACT instruction count halves)
                    ps = psS.tile(
                        [128, 2, 512], F32, tag="s", name=f"s{ch}_{kt}"
                    )
                    c, ks = b * QT_PER_S + kt // 4, kt % 4
                    for h in range(HP):
                        nc.tensor.matmul(
                            ps[:, h, off:512],
                            kT_sb[
                                64 * h : 64 * (h + 1),
                                c,
                                128 * ks : 128 * (ks + 1),
                            ],
                            qT_sb[64 * h : 64 * (h + 1), ch, off:512],
                            start=True,
                            stop=True,
                        )
                    pt = pp.tile([128, 2, 512], BF16, tag="p")
                    nc.scalar.activation(
                        pt[:, :, off:512], ps[:, :, off:512], AF.Exp,
                        scale=float(SCALE),
                    )
                    if mode == "causal" and 0 <= kt - 4 * qt < 4:
                        for h in range(HP):
                            # boundary block: triangular mask in place
                            nc.vector.tensor_tensor(
                                pt[:, h, off : off + 128],
                                pt[:, h, off : off + 128],
                                tri_sb[:],
                                mybir.AluOpType.mult,
                            )
                    if mode == "general":
                        for h in range(HP):
                            nc.vector.tensor_tensor(
                                pt[:, h, :],
                                pt[:, h, :],
                                mt_sb[:, qt * KT_PER_S + kt, :],
                                mybir.AluOpType.mult,
                            )
                    return pt

                def emit_av(kt, off, pt, first, last):
                    for h in range(HP):
                        nc.tensor.matmul(
                            av[h][0:65, off:512],
                            v_sb[:, b * KT_PER_S + kt, h, 0:65],
                            pt[:, h, off:512],
                            start=first,
                            stop=last,
                            skip_group_check=True,
                        )

                # software-pipelined: AV trails scores by one k-tile so the
                # ScalarE exp has a full matmul-pair of slack.
                prev = None
                for i, (kt, off) in enumerate(seq):
                    pt = emit_scores(kt, off)
                    if prev is not None:
                        emit_av(*prev, first=(i == 1), last=False)
                    prev = (kt, off, pt)
                emit_av(*prev, first=(len(seq) == 1), last=True)

                # epilogue: normalize. reciprocal on DVE; broadcast across
                # partitions via a ones-column PE matmul into a spare PSUM
                # slot (no DRAM round trip, so the DVE never head-of-line
                # blocks on a DMA and the a2a staging fires promptly).
                attnT = misc.tile([128, 512], BF16, tag="attnT", bufs=4)
                for h in range(HP):
                    rec = misc.tile([1, 512], BF16, tag="r", bufs=4)
                    with nc.allow_low_precision(
                        reason="bf16 softmax denominators; 2e-2 rel tolerance"
                    ):
                        nc.vector.reciprocal(rec[:], av[h][64:65, :])
                    psb = psT.tile([128, 512], F32, tag="t", name=f"rb{ch}_{h}")
                    nc.tensor.matmul(
                        psb[0:64, :], ones_sb[:], rec[:], start=True, stop=True
                    )
                    rb = misc.tile([64, 512], BF16, tag="rb", bufs=4)
                    nc.vector.tensor_copy(rb[:], psb[0:64, :])
                    nc.vector.tensor_tensor(
                        attnT[64 * h : 64 * (h + 1), :],
                        av[h][0:64, :],
                        rb[:],
                        mybir.AluOpType.mult,
                    )
                if skip_phase3:
                    return
                nc.sync.dma_start(a2a_in[b][2 * qt, :, :], attnT[:, 0:256])
                nc.sync.dma_start(
                    a2a_in[b][2 * qt + 1, :, :], attnT[:, 256:512]
                )

            def a2a(b):
                nc.gpsimd.collective_compute(
                    "AllToAll",
                    mybir.AluOpType.bypass,
                    replica_groups=[list(range(NCORES))],
                    ins=[a2a_in[b].opt()],
                    outs=[a2a_out[b].opt()],
                )

            def outproj(b):
                ab = const.tile([128, 8, 256], BF16, name=f"attn_all{b}")
                for r in range(NCORES):
                    nc.sync.dma_start(ab[:, r, :], a2a_out[b][r, :, :])
                for st2 in range(2):
                    for half in range(2):
                        pso = psT.tile(
                            [128, 512], F32, tag="t", name=f"o{b}_{st2}_{half}"
                        )
                        for r in range(NCORES):
                            nc.tensor.matmul(
                                pso[:],
                                ab[:, r, 128 * st2 : 128 * (st2 + 1)],
                                wo_sb[:, r, 512 * half : 512 * (half + 1)],
                                start=(r == 0),
                                stop=(r == NCORES - 1),
                            )
                        osb = misc.tile([128, 512], F32, tag="osb", bufs=4)
                        nc.vector.tensor_copy(osb[:], pso[:])
                        nc.sync.dma_start(
                            out[
                                256 * b + 128 * st2 : 256 * b + 128 * (st2 + 1),
                                512 * half : 512 * (half + 1),
                            ],
                            osb[:],
                        )

            # ---- emission: interleave projection chunks with attention so
            # the PE stream stays dense; wo loads early; a2a(b) fires right
            # after batch b's last attention chunk stages.
            qkv_chunk(0)
            qkv_chunk(1)
            nc.sync.dma_start(
                wo_sb[:], wo.rearrange("(o p) n -> p o n", p=128)
            )
            attention(0, 0)
            qkv_chunk(2)
            attention(0, 1)
            qkv_chunk(3)
            attention(0, 2)
            qkv_chunk(4)
            attention(0, 3)
            qkv_chunk(5)
            if not skip_phase3:
                a2a(0)
            attention(1, 0)
            qkv_chunk(6)
            attention(1, 1)
            qkv_chunk(7)
            attention(1, 2)
            attention(1, 3)
            if not skip_phase3:
                a2a(1)
                outproj(0)
                outproj(1)
            else:
                dbg = misc.tile([128, 512], F32, tag="attnT")
                nc.vector.tensor_copy(dbg[:], qT_sb[:, 0, :])
                nc.sync.dma_start(out[0:128, 0:512], dbg[:])
    _split_waits(nc)

    # The libneuronxla NEFF cache hashes the HLO, but the BIR travels in
    # backend_config which is NOT part of the hash — two different kernels
    # with identical I/O signatures collide and the stale NEFF runs. Encode
    # a hash of the BIR into the shape of an unused dummy input so the HLO
    # (and therefore the cache key) changes whenever the kernel changes.
    import hashlib

    hv = int.from_bytes(
        hashlib.sha256(nc.to_json_bytes()).digest()[:4], "little"
    )
    nonce_shape = [hv % 1021 + 1, (hv // 1021) % 1021 + 1]
    nc.dram_tensor("nonce", nonce_shape, F32, kind="ExternalInput")
    nc._nonce_shape = nonce_shape
    return nc


_BUILD_CACHE = {}


def _get_nc(mode, n_mask_tiles):
    key = (mode, n_mask_tiles)
    if key not in _BUILD_CACHE:
        _BUILD_CACHE[key] = build(mode, n_mask_tiles)
    return _BUILD_CACHE[key]


def kernel(x, Wqkv, Wo, mask):
    x = np.asarray(x)
    Wqkv = np.asarray(Wqkv)
    Wo = np.asarray(Wo)
    mask = np.asarray(mask)

    m2 = mask.reshape(S, S)
    if np.array_equal(m2, np.tril(np.ones((S, S), bool))):
        mode = "causal"
    elif m2.all():
        mode = "full"
    else:
        mode = "general"

    # host-side input prep: transpose x, slice per-head weight shards
    xT = np.ascontiguousarray(x.reshape(T, DM).T).astype(ml_dtypes.bfloat16)
    w4 = Wqkv.reshape(DM, H, 3, D)

    if mode == "general":
        tiles = []
        for qt in range(QT_PER_S):
            for kt in range(KT_PER_S):
                sub = m2[512 * qt : 512 * (qt + 1), 128 * kt : 128 * (kt + 1)]
                tiles.append(sub.T)
        mts = np.stack(tiles).astype(ml_dtypes.bfloat16)
        n_mask_tiles = len(tiles)
    else:
        mts = None
        n_mask_tiles = 0

    nc = _get_nc(mode, n_mask_tiles)

    in_maps = []
    for j in range(NCORES):
        hs = slice(HP * j, HP * (j + 1))
        im = {
            "xT": xT,
            "wq": np.ascontiguousarray(w4[:, hs, 0, :].reshape(DM, HP * D))
            .astype(ml_dtypes.bfloat16),
            "wk": np.ascontiguousarray(w4[:, hs, 1, :].reshape(DM, HP * D))
            .astype(ml_dtypes.bfloat16),
            "wv": np.ascontiguousarray(w4[:, hs, 2, :].reshape(DM, HP * D))
            .astype(ml_dtypes.bfloat16),
            "wo": Wo.astype(ml_dtypes.bfloat16),
            "nonce": np.zeros(nc._nonce_shape, np.float32),
        }
        if mode == "causal":
            # boundary block mask: valid (1.0) where q_rel >= k
            kk = np.arange(128)[:, None]
            qq = np.arange(128)[None, :]
            im["tri"] = (qq >= kk).astype(ml_dtypes.bfloat16)
        if n_mask_tiles:
            im["mt"] = mts
        in_maps.append(im)

    res = run_bass_kernel_spmd(nc, in_maps, list(range(NCORES)))
    # core j's output rows: [0:256] = batch 0 tokens [256j, 256j+256),
    #                       [256:512] = batch 1 tokens [256j, 256j+256)
    full = np.empty((B, S, DM), np.float32)
    for j in range(NCORES):
        o = res.results[j]["out"]
        for b in range(B):
            full[b, 256 * j : 256 * (j + 1), :] = o[256 * b : 256 * (b + 1)]
    return full


if __name__ == "__main__":
    rng = np.random.default_rng(0)
    x = rng.standard_normal((B, S, DM), dtype=np.float32)
    Wqkv = rng.standard_normal((DM, 3 * H * D), dtype=np.float32) * DM**-0.5
    Wo = rng.standard_normal((H * D, DM), dtype=np.float32) * (H * D) ** -0.5
    mask = np.tril(np.ones((S, S), bool))[None, None]
    out = kernel(x=x, Wqkv=Wqkv, Wo=Wo, mask=mask)
    print(out.shape, out.dtype)


# revision 37
# speedup vs baseline: 1.2603x; 1.2603x over previous
"""Multi-head causal attention on 8 Trainium2 NeuronCores.

Sharding: tensor-parallel over heads (2 heads/core) for QKV projection and
attention; All-to-All converts to token-sharding (512 tokens/core) for the
output projection, so each core writes a disjoint output slice and the host
gather is pure concatenation.

v2 layout strategy (per core), all-bf16 data plane:
  - qT/kT = W^T x^T computed in transposed [feature, token] layout (x^T in
    bf16 prepared on host); V computed directly in token-major layout with
    x^T tiles as the stationary operand (no PE transposes).
  - scores^T[k, q] = K_tile^T.T @ Q^T, two heads row-tiled on the PE.
    Causal diagonal tiles shrink to their valid q-suffix (saves ~15% PE+ACT),
    with one [128,128] triangular bf16 mask for the boundary block.
  - softmax: exp on ScalarE out of PSUM with 1/sqrt(D) folded into the
    activation scale; denominator via a ones-column appended to V.
  - normalization: reciprocal (DVE) -> partition_broadcast + multiply on the
    otherwise-idle GpSimd engine (no DRAM round trip, no DVE head-of-line
    blocking).
  - A2A payload bf16 (halves link traffic); output projection streams Wo
    (preloaded early) against per-row-DMA'd A2A results, PSUM -> DRAM direct.
"""

import numpy as np
import ml_dtypes

import concourse.bass as bass
import concourse.mybir as mybir
import concourse.tile as tile
from concourse.bass_utils import run_bass_kernel_spmd
from concourse.masks import make_identity

F32 = mybir.dt.float32
BF16 = mybir.dt.bfloat16
AF = mybir.ActivationFunctionType


def _install_cache_nonce_hook():
    """The libneuronxla NEFF cache hashes the HLO but the BIR rides in
    backend_config (excluded from the hash), so edited kernels with the same
    I/O signature can silently hit a stale cached NEFF. Inject a hash of the
    BIR into mhlo.frontend_attributes — which IS part of the model hash —
    the same way bass2jax ships the DVE tables."""
    import hashlib
    import concourse.bass2jax as bass2jax
    from jax.interpreters import mlir

    if getattr(bass2jax, "_ant_cache_nonce_hooked", False):
        return
    bass2jax._ant_cache_nonce_hooked = True
    orig = bass2jax._accumulate_module_dve_attrs

    def patched(ctx, nc):
        orig(ctx, nc)
        op = ctx.module_context.module.operation
        cur = (
            op.attributes["mhlo.frontend_attributes"]
            if "mhlo.frontend_attributes" in op.attributes
            else None
        )
        existing = (
            {a.name: mlir.ir.StringAttr(a.attr).value for a in cur}
            if cur is not None
            else {}
        )
        existing["ant.cache_nonce"] = hashlib.sha256(
            nc.to_json_bytes()
        ).hexdigest()
        op.attributes["mhlo.frontend_attributes"] = mlir.ir.DictAttr.get(
            {k: mlir.ir.StringAttr.get(v) for k, v in existing.items()}
        )

    bass2jax._accumulate_module_dve_attrs = patched


_install_cache_nonce_hook()


def _install_ldw_opt_hook():
    """bass_utils hardcodes --enable-ldw-opt=false. Flipping it to true fails
    in THIS walrus build even for a minimal all-bf16 single-matmul kernel
    ("InstLdweights is not compatible with LDW optimization" from
    visitInstLdweights) — the pass expects to synthesize Ldweights itself and
    rejects bass's explicit ones. Kept for reference; do NOT install."""
    import concourse.bass_utils as bu

    if getattr(bu, "_ant_ldw_opt_hooked", False):
        return
    bu._ant_ldw_opt_hooked = True
    orig = bu.run_command

    def patched(argv, **kwargs):
        argv = [
            "--enable-ldw-opt=true" if a == "--enable-ldw-opt=false" else a
            for a in argv
        ]
        return orig(argv, **kwargs)

    bu.run_command = patched


B, S, DM = 2, 2048, 1024
H, D = 16, 64
NCORES = 8
HP = H // NCORES          # heads per core
T = B * S                 # 4096 tokens
TCHUNK = T // NCORES      # 512 tokens per a2a chunk
NCH = T // 512            # 8 token chunks of 512
KT_PER_S = S // 128       # 16 k-tiles per sequence
QT_PER_S = S // 512       # 4 q-tiles per sequence
SCALE = 1.0 / np.sqrt(D)


MAX_WAITS = 1  # walrus in this container rejects >1 sem-wait per instruction


def _split_waits(nc, limit=MAX_WAITS):
    """Post-pass: move excess sem-waits onto preceding same-engine nops.

    Engines dispatch in program order and a sem-wait stalls the engine's NX
    before anything later is enqueued, so carrying the waits on nops placed
    immediately before the instruction is semantically identical.
    """
    n_id = 0
    for bb in nc.main_func.blocks:
        new = []
        for inst in bb.instructions:
            si = getattr(inst, "sync_info", None)
            # walrus --enable-ldw-opt fuses each Ldweights into the previous
            # matmul's pipeline and rejects any Ldweights carrying sem-waits;
            # move ALL of its waits onto nops (semantically identical: the
            # engine stalls just before instead of at the Ldweights).
            keep = 0 if isinstance(inst, mybir.InstLdweights) else limit
            if si is not None and len(si.on_wait) > keep:
                waits = list(si.on_wait)
                kept = waits[len(waits) - keep :] if keep else []
                move = waits[: len(waits) - keep]
                for i in range(0, len(move), limit):
                    nop = mybir.InstNoOp(
                        name=f"wsplit-{n_id}", ins=[], outs=[], engine=inst.engine
                    )
                    n_id += 1
                    nop.sync_info = mybir.SyncInfo(
                        on_wait=move[i : i + limit], on_update=[]
                    )
                    new.append(nop)
                inst.sync_info = mybir.SyncInfo(
                    on_wait=kept, on_update=list(si.on_update)
                )
            new.append(inst)
        bb.instructions = new


from concourse.vector_clock import ScopedClock


class _TileCtx(tile.TileContext):
    """Work around a walrus codegen limit: the stock tail drain carries one
    sem-wait per (engine, DMA-lane), but this compiler build rejects >1-2
    waits on a Drain ("Too many sync wait commands"). Put each wait on its
    own SP nop between the drain and the final barrier instead."""

    def _drain_and_barrier(self, tick_clock, wait_clock):
        nc = self.nc
        drain_inst = nc.sync.drain()
        wait_clock.add_sem_waits(
            drain_inst.ins, ScopedClock({None: tick_clock.global_clock})
        )
        si = drain_inst.ins.sync_info
        if si is not None and len(si.on_wait) > 1:
            waits = list(si.on_wait)
            drain_inst.ins.sync_info = mybir.SyncInfo(
                on_wait=[waits[0]], on_update=list(si.on_update)
            )
            for w in waits[1:]:
                nop = nc.sync.nop(nofuse=True, hint="tail_drain_wait_split")
                nop.ins.sync_info = mybir.SyncInfo(on_wait=[w], on_update=[])

        nc.all_engine_barrier()
        assert self.sems is not None
        popped = nc._tile_sem_poison_stack.pop()
        assert popped is self._sem_poison
        nc.clear_and_free_semaphores(list(self.sems.allocated().values()))
        nc.all_engine_barrier()


def _kt_seq(qt, mode):
    """K-tile processing order for q-tile qt.

    Causal: diagonal group first (the o=0 member is full-width, so it can
    carry start=True for the whole-AV accumulation), then the earlier full
    tiles. Returns list of (kt, off) where off is the first valid q column.
    """
    if mode == "causal":
        seq = [(4 * qt + o, 128 * o) for o in range(4)]
        seq += [(kt, 0) for kt in range(4 * qt)]
        return seq
    return [(kt, 0) for kt in range(KT_PER_S)]


def build(mode, n_mask_tiles, skip_phase3=False):
    """Build the SPMD Bass program. mode: 'causal' | 'full' | 'general'."""
    nc = bass.Bass()

    xT = nc.dram_tensor("xT", [DM, T], BF16, kind="ExternalInput")
    wq = nc.dram_tensor("wq", [DM, 128], BF16, kind="ExternalInput")
    wk = nc.dram_tensor("wk", [DM, 128], BF16, kind="ExternalInput")
    wv = nc.dram_tensor("wv", [DM, 128], BF16, kind="ExternalInput")
    wo = nc.dram_tensor("wo", [DM, DM], BF16, kind="ExternalInput")
    if mode == "causal":
        tri = nc.dram_tensor("tri", [128, 128], BF16, kind="ExternalInput")
    if n_mask_tiles:
        mt = nc.dram_tensor(
            "mt", [n_mask_tiles, 128, 512], BF16, kind="ExternalInput"
        )
    out = nc.dram_tensor("out", [TCHUNK, DM], F32, kind="ExternalOutput")

    with _TileCtx(nc) as tc:
        with (
            tc.tile_pool(name="const", bufs=1) as const,
            tc.tile_pool(name="xin", bufs=1) as xin,
            tc.tile_pool(name="pp", bufs=6) as pp,
            tc.tile_pool(name="misc", bufs=4) as misc,
            tc.tile_pool(name="psS", bufs=2, space="PSUM") as psS,
            tc.tile_pool(name="psAV", bufs=2, space="PSUM") as psAV,
            tc.tile_pool(name="psT", bufs=2, space="PSUM") as psT,
            tc.tile_pool(name="dram", bufs=1, space="DRAM") as dram,
        ):
            # ---- resident SBUF tensors ----
            wq_sb = const.tile([128, 8, 128], BF16)
            wk_sb = const.tile([128, 8, 128], BF16)
            wv_sb = const.tile([128, 8, 128], BF16)
            for w_sb, w in ((wq_sb, wq), (wk_sb, wk), (wv_sb, wv)):
                src = w.rearrange("(o p) e -> p o e", p=128)
                nc.sync.dma_start(w_sb[:, 0:4, :], src[:, 0:4, :])
                nc.sync.dma_start(w_sb[:, 4:8, :], src[:, 4:8, :])

            if mode == "causal":
                tri_sb = const.tile([128, 128], BF16)
                nc.sync.dma_start(tri_sb[:], tri[:, :])
            if n_mask_tiles:
                mt_sb = const.tile([128, n_mask_tiles, 512], BF16)
                nc.sync.dma_start(mt_sb[:], mt.rearrange("m p q -> p m q"))

            qT_sb = const.tile([128, NCH, 512], BF16)
            kT_sb = const.tile([128, NCH, 512], BF16)
            # V in [token, feature] layout, per k-tile, per head:
            # [p=token%128, ktile, head, 128] where cols 0:64 = v and cols
            # 64:128 = 1.0 — the AV matmul then replicates the softmax
            # denominator across output partitions 64:128 for free (matmul
            # cost scales with the moving width, not output partitions),
            # which kills the single-lane reciprocal + PE-broadcast chain.
            v_sb = const.tile([128, T // 128, HP, 128], BF16)
            nc.vector.memset(v_sb[:, :, :, 64:128], 1.0)
            wo_sb = const.tile([128, 8, DM], BF16)
            ident = const.tile([128, 128], F32)
            make_identity(nc, ident[:])

            a2a_in = [
                dram.tile([NCORES, 128, 256], BF16, name=f"a2a_in{b}")
                for b in range(B)
            ]
            a2a_out = [
                dram.tile([NCORES, 128, 256], BF16, name=f"a2a_out{b}")
                for b in range(B)
            ]

            # all of x^T preloaded up front (fits SBUF at bf16 when no mask
            # tiles are resident): the DMA burst is concentrated at kernel
            # start instead of taxing the PE's SBUF reads throughout the
            # whole QKV phase.
            preload = not n_mask_tiles
            xts = [None] * NCH

            def load_xt(c):
                xt = xin.tile(
                    [128, 8, 512], BF16,
                    tag=f"xt{c}" if preload else "xt",
                    bufs=1 if preload else 2,
                    name=f"xt{c}",
                )
                src = xT[:, 512 * c : 512 * (c + 1)].rearrange(
                    "(o p) s -> p o s", p=128
                )
                nc.sync.dma_start(xt[:, 0:4, :], src[:, 0:4, :])
                nc.sync.dma_start(xt[:, 4:8, :], src[:, 4:8, :])
                return xt

            if preload:
                for c in range(NCH):
                    xts[c] = load_xt(c)

            def qkv_chunk(c):
                xt = xts[c] if preload else load_xt(c)
                # q, k, v: weight-stationary, [feature, token] layout.
                # (One 512-wide matmul per contraction step beats a token-major
                # V: the PE is instruction-rate-bound, not row-bound.)
                for name, w_sb, dst in (
                    ("q", wq_sb, qT_sb),
                    ("k", wk_sb, kT_sb),
                    ("v", wv_sb, None),
                ):
                    psum = psT.tile(
                        [128, 512], F32, tag="t", name=f"ps_{name}{c}"
                    )
                    for kt in range(8):
                        nc.tensor.matmul(
                            psum[:],
                            w_sb[:, kt, :],
                            xt[:, kt, :],
                            start=(kt == 0),
                            stop=(kt == 7),
                        )
                    if dst is not None:
                        nc.vector.tensor_copy(dst[:, c, :], psum[:])
                        continue
                    # V: PE-transpose [feature, token] -> [token, feature].
                    # f32 throughout: a bf16 PSUM transpose output compiles
                    # but takes down the exec unit at runtime. Copies run as
                    # ScalarE activation-copies (Pool can't read PSUM) so the
                    # in-order DVE (busy with q/k copies + attention
                    # epilogues) never gates the psT ring.
                    vstg = misc.tile([128, 512], F32, tag="vstg", bufs=2)
                    nc.scalar.activation(vstg[:], psum[:], AF.Copy)
                    ps_t = psT.tile([128, 512], F32, tag="t", name=f"ps_t{c}")
                    for sub in range(4):
                        nc.tensor.transpose(
                            ps_t[:, 128 * sub : 128 * (sub + 1)],
                            vstg[:, 128 * sub : 128 * (sub + 1)],
                            ident[:],
                        )
                    for sub in range(4):
                        ktile = 4 * c + sub
                        for h in range(HP):
                            nc.scalar.activation(
                                v_sb[:, ktile, h, 0:64],
                                ps_t[
                                    :,
                                    128 * sub + 64 * h : 128 * sub
                                    + 64 * (h + 1),
                                ],
                                AF.Copy,
                            )

            def attention(b, qt):
                ch = b * QT_PER_S + qt
                seq = _kt_seq(qt, mode)
                av = [
                    psAV.tile([128, 512], F32, tag="av", name=f"av{ch}_{h}")
                    for h in range(HP)
                ]

                def emit_scores(kt, off):
                    # both heads share one [128, 2, 512] PSUM tile so a single
                    # ScalarE exp covers them (ACT instruction count halves)
                    ps = psS.tile(
                        [128, 2, 512], F32, tag="s", name=f"s{ch}_{kt}"
                    )
                    c, ks = b * QT_PER_S + kt // 4, kt % 4
                    for h in range(HP):
                        nc.tensor.matmul(
                            ps[:, h, off:512],
                            kT_sb[
                                64 * h : 64 * (h + 1),
                                c,
                                128 * ks : 128 * (ks + 1),
                            ],
                            qT_sb[64 * h : 64 * (h + 1), ch, off:512],
                            start=True,
                            stop=True,
                        )
                    pt = pp.tile([128, 2, 512], BF16, tag="p")
                    nc.scalar.activation(
                        pt[:, :, off:512], ps[:, :, off:512], AF.Exp,
                        scale=float(SCALE),
                    )
                    if mode == "causal" and 0 <= kt - 4 * qt < 4:
                        for h in range(HP):
                            # boundary block: triangular mask in place
                            nc.vector.tensor_tensor(
                                pt[:, h, off : off + 128],
                                pt[:, h, off : off + 128],
                                tri_sb[:],
                                mybir.AluOpType.mult,
                            )
                    if mode == "general":
                        for h in range(HP):
                            nc.vector.tensor_tensor(
                                pt[:, h, :],
                                pt[:, h, :],
                                mt_sb[:, qt * KT_PER_S + kt, :],
                                mybir.AluOpType.mult,
                            )
                    return pt

                def emit_av(kt, off, pt, first, last):
                    for h in range(HP):
                        nc.tensor.matmul(
                            av[h][0:128, off:512],
                            v_sb[:, b * KT_PER_S + kt, h, 0:128],
                            pt[:, h, off:512],
                            start=first,
                            stop=last,
                            skip_group_check=True,
                        )

                # software-pipelined: AV trails scores by one k-tile so the
                # ScalarE exp has a full matmul-pair of slack.
                prev = None
                for i, (kt, off) in enumerate(seq):
                    pt = emit_scores(kt, off)
                    if prev is not None:
                        emit_av(*prev, first=(i == 1), last=False)
                    prev = (kt, off, pt)
                emit_av(*prev, first=(len(seq) == 1), last=True)

                # epilogue: normalize. reciprocal on DVE; broadcast across
                # partitions via a ones-column PE matmul into a spare PSUM
                # slot (no DRAM round trip, so the DVE never head-of-line
                # blocks on a DMA and the a2a staging fires promptly).
                attnT = misc.tile([128, 512], BF16, tag="attnT", bufs=4)
                for h in range(HP):
                    # 1/denom as exp(-ln(d)) on ScalarE over the replicated
                    # [64,512] block: every step runs on 64+ partitions (the
                    # stock DVE reciprocal on the [1,512] row ran on a single
                    # lane and took ~3.3us on the critical chain). Table
                    # accuracy ~1e-3 rel, well inside the 2e-2 budget.
                    lnd = misc.tile([64, 512], F32, tag="lnd", bufs=4)
                    nc.scalar.activation(lnd[:], av[h][64:128, :], AF.Ln)
                    rb = misc.tile([64, 512], BF16, tag="rb", bufs=4)
                    nc.scalar.activation(rb[:], lnd[:], AF.Exp, scale=-1.0)
                    nc.vector.tensor_tensor(
                        attnT[64 * h : 64 * (h + 1), :],
                        av[h][0:64, :],
                        rb[:],
                        mybir.AluOpType.mult,
                    )
                if skip_phase3:
                    return
                nc.sync.dma_start(a2a_in[b][2 * qt, :, :], attnT[:, 0:256])
                nc.sync.dma_start(
                    a2a_in[b][2 * qt + 1, :, :], attnT[:, 256:512]
                )

            def a2a(b):
                nc.gpsimd.collective_compute(
                    "AllToAll",
                    mybir.AluOpType.bypass,
                    replica_groups=[list(range(NCORES))],
                    ins=[a2a_in[b].opt()],
                    outs=[a2a_out[b].opt()],
                )

            def outproj(b):
                ab = const.tile([128, 8, 256], BF16, name=f"attn_all{b}")
                for r in range(NCORES):
                    nc.sync.dma_start(ab[:, r, :], a2a_out[b][r, :, :])
                for st2 in range(2):
                    for half in range(2):
                        pso = psT.tile(
                            [128, 512], F32, tag="t", name=f"o{b}_{st2}_{half}"
                        )
                        for r in range(NCORES):
                            nc.tensor.matmul(
                                pso[:],
                                ab[:, r, 128 * st2 : 128 * (st2 + 1)],
                                wo_sb[:, r, 512 * half : 512 * (half + 1)],
                                start=(r == 0),
                                stop=(r == NCORES - 1),
                            )
                        osb = misc.tile([128, 512], F32, tag="osb", bufs=4)
                        nc.vector.tensor_copy(osb[:], pso[:])
                        nc.sync.dma_start(
                            out[
                                256 * b + 128 * st2 : 256 * b + 128 * (st2 + 1),
                                512 * half : 512 * (half + 1),
                            ],
                            osb[:],
                        )

            # ---- emission: interleave projection chunks with attention so
            # the PE stream stays dense; wo loads early; a2a(b) fires right
            # after batch b's last attention chunk stages.
            qkv_chunk(0)
            qkv_chunk(1)
            nc.sync.dma_start(
                wo_sb[:], wo.rearrange("(o p) n -> p o n", p=128)
            )
            attention(0, 0)
            qkv_chunk(2)
            attention(0, 1)
            qkv_chunk(3)
            attention(0, 2)
            qkv_chunk(4)
            attention(0, 3)
            qkv_chunk(5)
            if not skip_phase3:
                a2a(0)
            attention(1, 0)
            qkv_chunk(6)
            attention(1, 1)
            qkv_chunk(7)
            attention(1, 2)
            attention(1, 3)
            if not skip_phase3:
                a2a(1)
                outproj(0)
                outproj(1)
            else:
                dbg = misc.tile([128, 512], F32, tag="attnT")
                nc.vector.tensor_copy(dbg[:], qT_sb[:, 0, :])
                nc.sync.dma_start(out[0:128, 0:512], dbg[:])
    _split_waits(nc)

    # The libneuronxla NEFF cache hashes the HLO, but the BIR travels in
    # backend_config which is NOT part of the hash — two different kernels
    # with identical I/O signatures collide and the stale NEFF runs. Encode
    # a hash of the BIR into the shape of an unused dummy input so the HLO
    # (and therefore the cache key) changes whenever the kernel changes.
    import hashlib

    hv = int.from_bytes(
        hashlib.sha256(nc.to_json_bytes()).digest()[:4], "little"
    )
    nonce_shape = [hv % 1021 + 1, (hv // 1021) % 1021 + 1]
    nc.dram_tensor("nonce", nonce_shape, F32, kind="ExternalInput")
    nc._nonce_shape = nonce_shape
    return nc


_BUILD_CACHE = {}


def _get_nc(mode, n_mask_tiles):
    key = (mode, n_mask_tiles)
    if key not in _BUILD_CACHE:
        _BUILD_CACHE[key] = build(mode, n_mask_tiles)
    return _BUILD_CACHE[key]


def kernel(x, Wqkv, Wo, mask):
    x = np.asarray(x)
    Wqkv = np.asarray(Wqkv)
    Wo = np.asarray(Wo)
    mask = np.asarray(mask)

    m2 = mask.reshape(S, S)
    if np.array_equal(m2, np.tril(np.ones((S, S), bool))):
        mode = "causal"
    elif m2.all():
        mode = "full"
    else:
        mode = "general"

    # host-side input prep: transpose x, slice per-head weight shards
    xT = np.ascontiguousarray(x.reshape(T, DM).T).astype(ml_dtypes.bfloat16)
    w4 = Wqkv.reshape(DM, H, 3, D)

    if mode == "general":
        tiles = []
        for qt in range(QT_PER_S):
            for kt in range(KT_PER_S):
                sub = m2[512 * qt : 512 * (qt + 1), 128 * kt : 128 * (kt + 1)]
                tiles.append(sub.T)
        mts = np.stack(tiles).astype(ml_dtypes.bfloat16)
        n_mask_tiles = len(tiles)
    else:
        mts = None
        n_mask_tiles = 0

    nc = _get_nc(mode, n_mask_tiles)

    in_maps = []
    for j in range(NCORES):
        hs = slice(HP * j, HP * (j + 1))
        im = {
            "xT": xT,
            "wq": np.ascontiguousarray(w4[:, hs, 0, :].reshape(DM, HP * D))
            .astype(ml_dtypes.bfloat16),
            "wk": np.ascontiguousarray(w4[:, hs, 1, :].reshape(DM, HP * D))
            .astype(ml_dtypes.bfloat16),
            "wv": np.ascontiguousarray(w4[:, hs, 2, :].reshape(DM, HP * D))
            .astype(ml_dtypes.bfloat16),
            "wo": Wo.astype(ml_dtypes.bfloat16),
            "nonce": np.zeros(nc._nonce_shape, np.float32),
        }
        if mode == "causal":
            # boundary block mask: valid (1.0) where q_rel >= k
            kk = np.arange(128)[:, None]
            qq = np.arange(128)[None, :]
            im["tri"] = (qq >= kk).astype(ml_dtypes.bfloat16)
        if n_mask_tiles:
            im["mt"] = mts
        in_maps.append(im)

    res = run_bass_kernel_spmd(nc, in_maps, list(range(NCORES)))
    # core j's output rows: [0:256] = batch 0 tokens [256j, 256j+256),
    #                       [256:512] = batch 1 tokens [256j, 256j+256)
    full = np.empty((B, S, DM), np.float32)
    for j in range(NCORES):
        o = res.results[j]["out"]
        for b in range(B):
            full[b, 256 * j : 256 * (j + 1), :] = o[256 * b : 256 * (b + 1)]
    return full


if __name__ == "__main__":
    rng = np.random.default_rng(0)
    x = rng.standard_normal((B, S, DM), dtype=np.float32)
    Wqkv = rng.standard_normal((DM, 3 * H * D), dtype=np.float32) * DM**-0.5
    Wo = rng.standard_normal((H * D, DM), dtype=np.float32) * (H * D) ** -0.5
    mask = np.tril(np.ones((S, S), bool))[None, None]
    out = kernel(x=x, Wqkv=Wqkv, Wo=Wo, mask=mask)
    print(out.shape, out.dtype)
